# revision 9
# baseline (speedup 1.0000x reference)
"""HDC encoder kernel (embedding_lookup) for 8 TRN2 NeuronCores.

Computes, for inputs x (16,28,28) in [0,1], position table pw (784,10000),
level table vw (256,10000):

    idx   = clip(round(x.reshape(16,784) * 255), 0, 255)
    s[b,:] = sum_p vw[idx[b,p],:] * pw[p,:]
    out    = where(s > 0, 1, -1)          # (16, 10000) f32

Sharding: tensor-parallel over the D=10000 axis across 8 cores (each core
owns 1250 columns of both tables; bind/bundle/sign are elementwise along D).

Per-core device pipeline:
  - load pw/vw column-slices with f32->bf16 cast-in-DMA (SWDGE)
  - stage vw slice to a DRAM scratch (256 x 1280 bf16, rows padded so
    row bytes % 256 == 0 as dma_gather requires)
  - per batch: dma_gather 784 rows -> (128, 7, 1280) tile, position p at
    partition p%128 / chunk p//128 (gather's native layout)
  - DVE bf16 tensor_mul (2x mode) with the matching pw layout
  - PE reduction over positions: stationary is a (128,16) all-zeros matrix
    with an all-ones column at position b, so batch b accumulates into
    row b of one (16,1280) f32 PSUM tile (single accumulation group)
  - ACT Sign(s - 0.5) drain (s is an exact integer; -0.5 maps s==0 -> -1)

All products are exactly +-1 in bf16 and PSUM accumulates in f32, so the
result is bit-exact vs the f32 reference.
"""

import numpy as np
from contextlib import ExitStack

import concourse.bass as bass
import concourse.mybir as mybir
import concourse.tile as tile
from concourse import bacc
from concourse.tile_rust import add_dep_helper

B = 16
P = 784
L = 256
D = 10000
NCORES = 8
DC = D // NCORES          # 1250 columns per core
DPAD = 1280               # padded gather row length (2560 B, % 256 == 0)
PC_FULL = P // 128        # 6 full 128-position chunks
PTAIL = P - PC_FULL * 128  # 16 positions in the last chunk
NCHUNKS = PC_FULL + 1     # 7
SW = P // 16              # 49 index columns per batch (16-partition wrap)
NSLICES = ((0, 512), (512, 1024), (1024, DPAD))  # PSUM-bank-aligned matmuls

BF16 = mybir.dt.bfloat16
F32 = mybir.dt.float32
I16 = mybir.dt.int16

LAST_RESULTS = None  # BassKernelResults of the most recent run (for test.py)


def build_nc(reps: int = 1):
    """Build the per-core program. reps>1 repeats the steady-state body
    (gather + multiply + reduce + drain) for wall-clock benchmarking."""
    nc = bacc.Bacc("TRN2", target_bir_lowering=False, debug=False)

    idx_d = nc.declare_dram_parameter("idx", [128, P], I16, isOutput=False)
    pw_d = nc.declare_dram_parameter("pw", [P, DC], F32, isOutput=False)
    vw_d = nc.declare_dram_parameter("vw", [L, DC], F32, isOutput=False)
    out_d = nc.declare_dram_parameter("out", [B, DC], F32, isOutput=True)
    vw_scratch = nc.dram_tensor("vw_scratch", [L, DPAD], BF16)

    with tile.TileContext(nc) as tc, ExitStack() as ctx:
        const_pool = ctx.enter_context(tc.tile_pool(name="const", bufs=1))
        gpool = ctx.enter_context(tc.tile_pool(name="g", bufs=3))
        ppool = ctx.enter_context(tc.tile_pool(name="prod", bufs=3))
        psum_pool = ctx.enter_context(
            tc.tile_pool(name="psum", bufs=1, space="PSUM")
        )

        pw_sb = const_pool.tile([128, NCHUNKS, DPAD], BF16)
        vw_sb = const_pool.tile([128, 2, DPAD], BF16)
        idx_sb = const_pool.tile([128, P], I16)
        zwin = const_pool.tile([128, 31], BF16)
        out_sb = const_pool.tile([B, DC], F32)
        bias_sb = const_pool.tile([B, 1], F32)
        psum = psum_pool.tile([B, DPAD], F32)

        # Zero the persistent staging tiles: covers the d-padding columns
        # [DC:DPAD) and the unused tail-chunk rows so products there are 0.
        nc.gpsimd.memset(pw_sb[:], 0.0)
        nc.gpsimd.memset(vw_sb[:], 0.0)
        nc.gpsimd.memset(zwin[:], 0.0)
        nc.gpsimd.memset(zwin[:, 15:16], 1.0)
        nc.gpsimd.memset(bias_sb[:], -0.5)

        # pw slice, cast f32->bf16, laid out to match the gather: position
        # p -> partition p%128, chunk p//128.
        nc.gpsimd.dma_start(
            out=pw_sb[:, 0:PC_FULL, 0:DC],
            in_=pw_d[0 : PC_FULL * 128].rearrange("(c p) d -> p c d", p=128),
        )
        nc.gpsimd.dma_start(
            out=pw_sb[0:PTAIL, PC_FULL, 0:DC], in_=pw_d[PC_FULL * 128 : P]
        )
        nc.gpsimd.dma_start(
            out=vw_sb[:, :, 0:DC], in_=vw_d.rearrange("(h p) d -> p h d", p=128)
        )
        nc.sync.dma_start(out=idx_sb[:], in_=idx_d[:])

        # Padded bf16 level table in DRAM for the row gather.
        st = nc.sync.dma_start(
            out=vw_scratch.rearrange("(h p) d -> p h d", p=128), in_=vw_sb[:]
        )

        for rep in range(reps):
            for b in range(B):
                g = gpool.tile([128, NCHUNKS, DPAD], BF16)
                gi = nc.gpsimd.dma_gather(
                    g[:],
                    vw_scratch[:],
                    idx_sb[:, b * SW : (b + 1) * SW],
                    P,
                    P,
                    DPAD,
                    elem_step=DPAD,
                )
                # The gather reads vw_scratch through DRAM; make the ordering
                # explicit in case DRAM RAW isn't tracked by the scheduler.
                add_dep_helper(gi.ins, st.ins, reason="gather reads vw_scratch")

                prod = ppool.tile([128, NCHUNKS, DPAD], BF16)
                nc.vector.tensor_mul(
                    prod[:, 0:PC_FULL, :],
                    g[:, 0:PC_FULL, :],
                    pw_sb[:, 0:PC_FULL, :],
                )
                nc.vector.tensor_mul(
                    prod[0:PTAIL, PC_FULL, :],
                    g[0:PTAIL, PC_FULL, :],
                    pw_sb[0:PTAIL, PC_FULL, :],
                )

                for c in range(NCHUNKS):
                    kk = 128 if c < PC_FULL else PTAIL
                    for n0, n1 in NSLICES:
                        nc.tensor.matmul(
                            psum[:, n0:n1],
                            zwin[0:kk, 15 - b : 31 - b],
                            prod[0:kk, c, n0:n1],
                            start=(b == 0 and c == 0),
                            stop=(b == B - 1 and c == NCHUNKS - 1),
                        )

            nc.scalar.activation(
                out_sb[:],
                psum[:, 0:DC],
                mybir.ActivationFunctionType.Sign,
                bias=bias_sb[:],
            )
            nc.sync.dma_start(out=out_d[:], in_=out_sb[:])

    nc.compile()
    return nc


def host_inputs(x, position_weight, value_weight):
    """Shard the full inputs into per-core in_maps."""
    x = np.asarray(x, dtype=np.float32).reshape(B, P)
    pw = np.ascontiguousarray(np.asarray(position_weight, dtype=np.float32))
    vw = np.ascontiguousarray(np.asarray(value_weight, dtype=np.float32))

    # Quantize to level indices on host (matches jnp round-half-to-even) and
    # wrap into dma_gather's index layout: index i of batch b sits at
    # [i % 16, b*49 + i//16] of a (128, 784) int16 tile.
    idx = np.clip(np.rint(x * (L - 1)), 0, L - 1).astype(np.int16)  # (16, 784)
    wrapped = idx.reshape(B, SW, 16).transpose(2, 0, 1).reshape(16, B * SW)
    # dma_gather reads the 16-partition-wrapped index block replicated per
    # GPSIMD core: core c reads partitions [16c, 16c+16).
    idx_arr = np.tile(wrapped, (8, 1))

    in_maps = []
    for c in range(NCORES):
        sl = slice(c * DC, (c + 1) * DC)
        in_maps.append(
            {
                "idx": idx_arr,
                "pw": np.ascontiguousarray(pw[:, sl]),
                "vw": np.ascontiguousarray(vw[:, sl]),
            }
        )
    return in_maps


def kernel(x, position_weight, value_weight):
    global LAST_RESULTS
    from concourse.bass_utils import run_bass_kernel_spmd
    import os

    nc = build_nc()
    in_maps = host_inputs(x, position_weight, value_weight)
    trace = bool(os.environ.get("BASS_TRACE"))
    res = run_bass_kernel_spmd(nc, in_maps, list(range(NCORES)), trace=trace)
    LAST_RESULTS = res
    out = np.concatenate([r["out"] for r in res.results], axis=1)
    return out.astype(np.float32)
